# revision 35
# baseline (speedup 1.0000x reference)
"""Trainium2 Bass kernel for explicit multi-head attention.

Problem: x[2, 2048, 1024], Wq/Wk/Wv/Wo[1024, 1024] (+biases), NHEAD=16.
Sharding over 8 NeuronCores: data-parallel on batch (cores 0-3 -> b=0,
cores 4-7 -> b=1), tensor-parallel over heads (4 heads per core).  Each
core computes its 4 heads' attention plus the partial out-projection
(ctx_local @ Wo[rows_local]); partials are summed on the host, which is
mathematically the all-reduce the sharding hint asks for.

Device-side layout notes:
 - Everything is computed transposed: xT [D, L] streams through the PE as
   the moving operand, producing qT/kT [d_local, L] and v [L, d_local].
 - Scores are computed transposed per (head, Lk-tile, Lq-chunk):
   sT = kT_tile^T-contraction -> [Lk=128, Lq=512]; softmax runs without
   max-subtraction (score range here is ~[-4, 4], exp is safe in fp32).
 - The softmax denominator is produced by the PV matmul itself: the
   stationary v tile carries an extra all-ones column, so one PSUM row
   accumulates sum_k exp(s).  Even head lands ctx in psum partitions
   0-63 / denom at 64; odd head (stationary window shifted) lands ctx at
   64-127 / denom at 32, so ctxT packs two heads per 128-partition tile
   with no cross-partition moves.
 - Normalization: DVE reciprocal of the denom row, an SBUF->SBUF DMA
   broadcasts it across partitions, and one DVE multiply fuses the
   normalize with the PSUM->SBUF evacuation (deferred one chunk so it
   overlaps the next chunk's matmuls).
 - Dtypes: attention operands bf16 (fast weight load), projections bf16
   inputs, out-projection float32r; PSUM accumulation is fp32 throughout.
 - Schedule: pair-1 q/k projections and the fused out-projection are
   drip-fed into the ACT-paced attention loops to keep the PE dense and
   the HAM clock-gate warm; dummy warm-up matmuls cover the DMA head.
"""

import os
import sys

import numpy as np

for _p in ("/opt/trn_rl_repo", "/root/.axon_site/_ro/trn_rl_repo"):
    if os.path.isdir(_p) and _p not in sys.path:
        sys.path.append(_p)

import concourse.bass as bass
import concourse.mybir as mybir
import concourse.tile as tile
from concourse import bacc
from concourse.bass_utils import run_bass_kernel_spmd

# ---------------------------------------------------------------------------
# ACT table-set pinning: this kernel's only activations are Identity, Exp and
# Ln, all present in the single "natural_log_exp_and_others" set.  The stock
# per-function chooser maps Exp to "exp_and_others" and Ln to the natural-log
# set, which alternates ACT_TABLE_LOADs (~2.6us each) every attention chunk.
# Restrict the function->set map so every activation resolves to the one set
# and exactly one table load is emitted.
import concourse.hw_specs as _hw_specs

_orig_get_activation_tables = _hw_specs.get_activation_tables


def _pinned_activation_tables(module_arch):
    t = _orig_get_activation_tables(module_arch)
    pin = {
        mybir.ActivationFunctionType.Exp,
        mybir.ActivationFunctionType.Ln,
        mybir.ActivationFunctionType.Identity,
        mybir.ActivationFunctionType.Copy,
    }
    out = {}
    for name, fns in t.items():
        if name == "natural_log_exp_and_others":
            out[name] = set(fns)
        else:
            out[name] = set(fns) - pin
    return out


bacc.get_activation_tables = _pinned_activation_tables

B = 2
L = 2048
D_MODEL = 1024
NHEAD = 16
D_HEAD = 64
SCALE = 1.0 / float(np.sqrt(D_HEAD))
N_CORES = 8
TP = 4                      # tensor-parallel group size (heads split)
HEADS_PER_CORE = NHEAD // TP          # 4
D_LOCAL = HEADS_PER_CORE * D_HEAD     # 256
N_PAIRS = HEADS_PER_CORE // 2         # 2 head-pairs per core
KT = D_MODEL // 128                   # 8 contraction tiles for projections
LT = L // 128                         # 16 L tiles
NCH = L // 512                        # 4 Lq chunks of 512

F32 = mybir.dt.float32
# PE matmul dtype: float32r = single-pass fp32 (full speed, slightly
# reduced multiply precision); float32 = exact two-pass (4x slower).
_MM_DT_NAME = os.environ.get("KERNEL_MM_DT", "f32r")
MM_DT = {"f32": mybir.dt.float32, "f32r": mybir.dt.float32r}[_MM_DT_NAME]
MMD = MM_DT  # dtype of SBUF tiles consumed by the main matmuls
# attention operands (qT/kT/v_aug/exp-probs) in bf16: same PE cycles/row as
# fp32r but fast-weight-load halves LDWEIGHTS, and SBUF footprint halves
ATT_DT = {"bf16": mybir.dt.bfloat16, "f32r": mybir.dt.float32r,
          "f32": mybir.dt.float32}[os.environ.get("KERNEL_ATT_DT", "bf16")]
# projection inputs (x, Wq/Wk/Wv, bv) in bf16: halves the HBM load (which
# paces the kernel head) at ~2e-3 relative error
PROJ_DT = {"bf16": mybir.dt.bfloat16, "f32r": mybir.dt.float32r,
           "f32": mybir.dt.float32}[os.environ.get("KERNEL_PROJ_DT", "bf16")]

# v_aug stationary layout (one tile per (pair, Lk-tile), [128, 193]):
#   cols 0:64    = v(even head)          -> even window cols 0:128
#   col  64      = ones (even denom)
#   cols 65:129  = zeros
#   col  97      = ones (odd denom; odd window col index 32)
#   cols 129:193 = v(odd head)           -> odd window cols 65:193
VAUG_W = 193


def _to_proj(x):
    if PROJ_DT is mybir.dt.bfloat16:
        import ml_dtypes

        return np.ascontiguousarray(np.asarray(x, np.float32).astype(ml_dtypes.bfloat16))
    return _round_fp32r(x)


def _round_fp32r(x):
    """Round fp32 to the fp32r grid (11-bit mantissa; low 12 bits zero) so a
    plain same-dtype DMA feeds PE-exact fp32r values."""
    if MM_DT is not mybir.dt.float32r:
        return np.ascontiguousarray(x, dtype=np.float32)
    b = np.ascontiguousarray(x, dtype=np.float32).view(np.uint32).astype(np.uint64)
    r = ((b + 0x7FF + ((b >> 12) & 1)) >> 12 << 12) & 0xFFFFFFFF
    return r.astype(np.uint32).view(np.float32)


def build_kernel():
    nc = bacc.Bacc("TRN2", target_bir_lowering=False, debug=False)

    xT = nc.dram_tensor("xT", [D_MODEL, L], PROJ_DT, kind="ExternalInput").ap()
    wq = nc.dram_tensor("wq", [D_MODEL, D_LOCAL], PROJ_DT, kind="ExternalInput").ap()
    wk = nc.dram_tensor("wk", [D_MODEL, D_LOCAL], PROJ_DT, kind="ExternalInput").ap()
    wv = nc.dram_tensor("wv", [D_MODEL, D_LOCAL], PROJ_DT, kind="ExternalInput").ap()
    wo = nc.dram_tensor("wo", [D_LOCAL, D_MODEL], MMD, kind="ExternalInput").ap()
    bq = nc.dram_tensor("bq", [D_LOCAL], F32, kind="ExternalInput").ap()
    bk = nc.dram_tensor("bk", [D_LOCAL], F32, kind="ExternalInput").ap()
    bv = nc.dram_tensor("bv", [D_LOCAL], PROJ_DT, kind="ExternalInput").ap()
    bo = nc.dram_tensor("bo", [D_MODEL], MMD, kind="ExternalInput").ap()
    out_p = nc.dram_tensor("out_p", [L, D_MODEL], F32, kind="ExternalOutput").ap()

    Exp = mybir.ActivationFunctionType.Exp
    Ln = mybir.ActivationFunctionType.Ln

    with tile.TileContext(nc) as tc:
        with (
            tc.tile_pool(name="persist", bufs=1) as persist,
            tc.tile_pool(name="exp_pool", bufs=3) as exp_pool,
            tc.tile_pool(name="recip_pool", bufs=2) as recip_pool,
            tc.tile_pool(name="bcs_pool", bufs=2) as bcs_pool,
            tc.tile_pool(name="out_pool", bufs=4) as out_pool,
            tc.tile_pool(name="ps_st", bufs=2, space="PSUM") as ps_st,
            tc.tile_pool(name="ps_ctx", bufs=4, space="PSUM") as ps_ctx,
        ):
            # ---- constants / weights (sync queue: v-path + xT first; the
            # scalar queue carries the rest in parallel) ----
            ones_sb = persist.tile([128, 128], MMD)
            nc.vector.memset(ones_sb[:].bitcast(F32), 1.0)
            ones_pj = persist.tile([1, 128], PROJ_DT)
            nc.vector.memset(
                ones_pj[:].bitcast(F32) if PROJ_DT == mybir.dt.float32r
                else ones_pj[:],
                1.0,
            )
            bv_sb = persist.tile([1, D_LOCAL], PROJ_DT)
            nc.sync.dma_start(bv_sb[:], bv.rearrange("(o n) -> o n", o=1))
            wv_sb = persist.tile([128, KT, D_LOCAL], PROJ_DT)
            nc.sync.dma_start(wv_sb[:], wv.rearrange("(k p) n -> p k n", p=128))
            xT_sb = persist.tile([128, KT, L], PROJ_DT)
            for k in range(KT):
                nc.sync.dma_start(xT_sb[:, k, :], xT[k * 128 : (k + 1) * 128, :])
            bq_sb = persist.tile([128, D_LOCAL // 128], F32)
            nc.scalar.dma_start(bq_sb[:], bq.rearrange("(m p) -> p m", p=128))
            bk_sb = persist.tile([128, D_LOCAL // 128], F32)
            nc.scalar.dma_start(bk_sb[:], bk.rearrange("(m p) -> p m", p=128))
            bo_sb = persist.tile([1, D_MODEL], MMD)
            nc.scalar.dma_start(bo_sb[:], bo.rearrange("(o n) -> o n", o=1))
            wq_sb = persist.tile([128, KT, D_LOCAL], PROJ_DT)
            nc.scalar.dma_start(wq_sb[:], wq.rearrange("(k p) n -> p k n", p=128))
            wk_sb = persist.tile([128, KT, D_LOCAL], PROJ_DT)
            nc.scalar.dma_start(wk_sb[:], wk.rearrange("(k p) n -> p k n", p=128))
            wo_sb = persist.tile([128, N_PAIRS, D_MODEL], MMD)
            nc.scalar.dma_start(wo_sb[:], wo.rearrange("(k p) n -> p k n", p=128))

            qT_sb = persist.tile([128, N_PAIRS, L], ATT_DT)
            kT_sb = persist.tile([128, N_PAIRS, L], ATT_DT)
            vaug = persist.tile([128, N_PAIRS, LT, VAUG_W], ATT_DT)
            ctxT_sb = persist.tile([128, N_PAIRS, L], MMD)

            _vm = (lambda ap: ap.bitcast(F32)) if ATT_DT == mybir.dt.float32r else (
                lambda ap: ap
            )
            nc.vector.memset(_vm(vaug[:, :, :, 64:129]), 0.0)
            nc.vector.memset(_vm(vaug[:, :, :, 64:65]), 1.0)
            nc.vector.memset(_vm(vaug[:, :, :, 97:98]), 1.0)

            # ---- v projection (natural layout; bias via K=1 rank-1) ----
            def emit_v(lt):
                ps = ps_ctx.tile([128, D_LOCAL], F32, tag="ctx")
                for k in range(KT):
                    nc.tensor.matmul(
                        ps[:],
                        xT_sb[:, k, lt * 128 : (lt + 1) * 128],
                        wv_sb[:, k, :],
                        start=(k == 0),
                        stop=False,
                    )
                nc.tensor.matmul(
                    ps[:], ones_pj[0:1, 0:128], bv_sb[0:1, :], start=False, stop=True
                )
                for p in range(N_PAIRS):
                    nc.vector.tensor_copy(
                        vaug[:, p, lt, 0:64], ps[:, p * 128 : p * 128 + 64]
                    )
                    nc.vector.tensor_copy(
                        vaug[:, p, lt, 129:193],
                        ps[:, p * 128 + 64 : p * 128 + 128],
                    )

            # ---- one qT/kT projection group: tensor t (0=q, 1=k), pair m,
            # Lq chunk c ----
            def emit_qk(t, m, c):
                w_sb, b_sb, dst = ((wq_sb, bq_sb, qT_sb), (wk_sb, bk_sb, kT_sb))[t]
                ps = ps_ctx.tile([128, 512], F32, tag="ctx")
                for k in range(KT):
                    nc.tensor.matmul(
                        ps[:],
                        w_sb[:, k, m * 128 : (m + 1) * 128],
                        xT_sb[:, k, c * 512 : (c + 1) * 512],
                        start=(k == 0),
                        stop=(k == KT - 1),
                    )
                nc.vector.tensor_scalar_add(
                    dst[:, m, c * 512 : (c + 1) * 512], ps[:], b_sb[:, m : m + 1]
                )

            def emit_normalize(p, c, ctx_e, ctx_o):
                # softmax denominators sit at psum partition 64 (even head)
                # / 32 (odd head).  1/d = exp(-ln d) on the Scalar engine
                # (both functions in one pinned ACT table set), an
                # SBUF->SBUF DMA broadcasts each row across partitions, and
                # one DVE multiply per head fuses normalize with the
                # PSUM->SBUF evacuation into the packed ctxT tile.
                rt = recip_pool.tile([128, 512], F32, tag="rt")
                nc.vector.reciprocal(rt[64:65, :], ctx_e[64:65, :])
                nc.vector.reciprocal(rt[32:33, :], ctx_o[32:33, :])
                bcs = bcs_pool.tile([128, 512], F32, tag="bcs")
                nc.sync.dma_start(
                    bcs[0:64, :],
                    rt[64:65, :].unsqueeze(1).broadcast_to([1, 64, 512]),
                )
                nc.sync.dma_start(
                    bcs[64:128, :],
                    rt[32:33, :].unsqueeze(1).broadcast_to([1, 64, 512]),
                )
                sl = slice(c * 512, (c + 1) * 512)
                nc.vector.tensor_mul(
                    ctxT_sb[0:64, p, sl], ctx_e[0:64, :], bcs[0:64, :]
                )
                nc.vector.tensor_mul(
                    ctxT_sb[64:128, p, sl], ctx_o[64:128, :], bcs[64:128, :]
                )

            def emit_outproj_tile(c, idx):
                # one [128, 512] tile of out[L, D] for Lq chunk c; psum
                # recycled from the ctx pool.
                m = 4 * c + idx // 2
                n = idx % 2
                po = ps_ctx.tile([128, 512], F32, tag="ctx")
                for k in range(N_PAIRS):
                    nc.tensor.matmul(
                        po[:],
                        ctxT_sb[:, k, m * 128 : (m + 1) * 128],
                        wo_sb[:, k, n * 512 : (n + 1) * 512],
                        start=(k == 0),
                        stop=False,
                    )
                nc.tensor.matmul(
                    po[:],
                    ones_sb[0:1, 0:128],
                    bo_sb[0:1, n * 512 : (n + 1) * 512],
                    start=False,
                    stop=True,
                )
                ot = out_pool.tile([128, 512], F32, tag="ot")
                nc.vector.tensor_copy(ot[:], po[:])
                nc.sync.dma_start(
                    out_p[m * 128 : (m + 1) * 128, n * 512 : (n + 1) * 512], ot[:]
                )

            # ---- emission schedule ----
            # PE warm-up: dense dummy matmuls on resident data while the
            # input DMAs land, so the HAM clock-gate reaches 2.4 GHz before
            # the real projections start (~3.4us of sustained PE activity).
            warm = ps_st.tile([128, 1024], F32, tag="sT")
            for i in range(40):
                nc.tensor.matmul(
                    warm[:, 0:128],
                    ones_sb[0:1, 0:128],
                    ones_sb[0:1, 0:128],
                    start=(i == 0),
                    stop=(i == 39),
                )

            # head: v projection + pair-0 q/k projections
            for lt in range(LT):
                emit_v(lt)
            for t in range(2):
                for c in range(NCH):
                    emit_qk(t, 0, c)

            # attention, pair-outer; pair-1 projections are drip-fed into
            # pair-0's attention (PE slack under the ACT-paced exp), and the
            # fused out-projection drips into pair-1's attention.
            # k-pair-1 groups first: (c0,p1) needs all kT columns but only
            # qT chunk c0; q-chunks land later in the drip
            qk_todo = [(1, 1, c) for c in range(NCH)] + [
                (0, 1, c) for c in range(NCH)
            ]
            outproj_q = []
            pending = None
            for c in range(NCH):
                for p in range(N_PAIRS):
                    ctx_e = ps_ctx.tile([128, 512], F32, tag="ctx")
                    ctx_o = ps_ctx.tile([128, 512], F32, tag="ctx")
                    if pending is not None:
                        # normalize the previous chunk now: all of it runs
                        # on DVE/DMA, so it overlaps this chunk's matmuls
                        # without costing the PE or ACT pacers anything
                        prev_p, prev_c = pending[0], pending[1]
                        emit_normalize(*pending)
                        pending = None
                        if prev_p == N_PAIRS - 1:
                            outproj_q.extend((prev_c, idx) for idx in range(8))
                    for j in range(LT):
                        sT = ps_st.tile([128, 1024], F32, tag="sT")
                        nc.tensor.matmul(
                            sT[:, 0:512],
                            kT_sb[0:64, p, j * 128 : (j + 1) * 128],
                            qT_sb[0:64, p, c * 512 : (c + 1) * 512],
                            start=True,
                            stop=True,
                        )
                        nc.tensor.matmul(
                            sT[:, 512:1024],
                            kT_sb[64:128, p, j * 128 : (j + 1) * 128],
                            qT_sb[64:128, p, c * 512 : (c + 1) * 512],
                            start=True,
                            stop=True,
                        )
                        et = exp_pool.tile([128, 1024], ATT_DT, tag="et")
                        nc.scalar.activation(et[:], sT[:], Exp, scale=SCALE)
                        nc.tensor.matmul(
                            ctx_e[:],
                            vaug[:, p, j, 0:128],
                            et[:, 0:512],
                            start=(j == 0),
                            stop=(j == LT - 1),
                        )
                        nc.tensor.matmul(
                            ctx_o[:],
                            vaug[:, p, j, 65:193],
                            et[:, 512:1024],
                            start=(j == 0),
                            stop=(j == LT - 1),
                        )
                        if c == 0 and p == 0 and j % 2 == 1 and qk_todo:
                            emit_qk(*qk_todo.pop(0))
                        if j % 2 == 0 and j >= 8 and outproj_q:
                            emit_outproj_tile(*outproj_q.pop(0))
                    pending = (p, c, ctx_e, ctx_o)
            emit_normalize(*pending)
            outproj_q.extend((NCH - 1, idx) for idx in range(8))
            for tile_ref in outproj_q:
                emit_outproj_tile(*tile_ref)

    nc.compile()
    return nc


_NC = None
LAST_RESULTS = None


def _get_nc():
    global _NC
    if _NC is None:
        _NC = build_kernel()
    return _NC


def kernel(x, Wq, bq, Wk, bk, Wv, bv, Wo, bo):
    global LAST_RESULTS
    x = np.asarray(x, dtype=np.float32)
    Wq = np.asarray(Wq, dtype=np.float32)
    Wk = np.asarray(Wk, dtype=np.float32)
    Wv = np.asarray(Wv, dtype=np.float32)
    Wo = np.asarray(Wo, dtype=np.float32)
    bq = np.asarray(bq, dtype=np.float32)
    bk = np.asarray(bk, dtype=np.float32)
    bv = np.asarray(bv, dtype=np.float32)
    bo = np.asarray(bo, dtype=np.float32)

    nc = _get_nc()

    xTb = [_to_proj(x[b].T) for b in range(B)]
    zeros_bo = np.zeros_like(bo)
    in_maps = []
    for c in range(N_CORES):
        b, tp = divmod(c, TP)
        sl = slice(tp * D_LOCAL, (tp + 1) * D_LOCAL)
        in_maps.append(
            {
                "xT": xTb[b],
                "wq": _to_proj(Wq[:, sl]),
                "wk": _to_proj(Wk[:, sl]),
                "wv": _to_proj(Wv[:, sl]),
                "wo": _round_fp32r(Wo[sl, :]),
                "bq": np.ascontiguousarray(bq[sl]),
                "bk": np.ascontiguousarray(bk[sl]),
                "bv": _to_proj(bv[sl]),
                "bo": _round_fp32r(bo) if tp == 0 else zeros_bo,
            }
        )

    res = run_bass_kernel_spmd(nc, in_maps, core_ids=list(range(N_CORES)))
    LAST_RESULTS = res

    out = np.empty((B, L, D_MODEL), dtype=np.float32)
    for b in range(B):
        acc = res.results[b * TP]["out_p"].astype(np.float32)
        for tp in range(1, TP):
            acc = acc + res.results[b * TP + tp]["out_p"]
        out[b] = acc
    return out


# revision 36
# speedup vs baseline: 1.0187x; 1.0187x over previous
"""Trainium2 Bass kernel for explicit multi-head attention.

Problem: x[2, 2048, 1024], Wq/Wk/Wv/Wo[1024, 1024] (+biases), NHEAD=16.
Sharding over 8 NeuronCores: data-parallel on batch (cores 0-3 -> b=0,
cores 4-7 -> b=1), tensor-parallel over heads (4 heads per core).  Each
core computes its 4 heads' attention plus the partial out-projection
(ctx_local @ Wo[rows_local]); partials are summed on the host, which is
mathematically the all-reduce the sharding hint asks for.

Device-side layout notes:
 - Everything is computed transposed: xT [D, L] streams through the PE as
   the moving operand, producing qT/kT [d_local, L] and v [L, d_local].
 - Scores are computed transposed per (head, Lk-tile, Lq-chunk):
   sT = kT_tile^T-contraction -> [Lk=128, Lq=512]; softmax runs without
   max-subtraction (score range here is ~[-4, 4], exp is safe in fp32).
 - The softmax denominator is produced by the PV matmul itself: the
   stationary v tile carries an extra all-ones column, so one PSUM row
   accumulates sum_k exp(s).  Even head lands ctx in psum partitions
   0-63 / denom at 64; odd head (stationary window shifted) lands ctx at
   64-127 / denom at 32, so ctxT packs two heads per 128-partition tile
   with no cross-partition moves.
 - Normalization: DVE reciprocal of the denom row, an SBUF->SBUF DMA
   broadcasts it across partitions, and one DVE multiply fuses the
   normalize with the PSUM->SBUF evacuation (deferred one chunk so it
   overlaps the next chunk's matmuls).
 - Dtypes: attention operands bf16 (fast weight load), projections bf16
   inputs, out-projection float32r; PSUM accumulation is fp32 throughout.
 - Schedule: pair-1 q/k projections and the fused out-projection are
   drip-fed into the ACT-paced attention loops to keep the PE dense and
   the HAM clock-gate warm; dummy warm-up matmuls cover the DMA head.
"""

import os
import sys

import numpy as np

for _p in ("/opt/trn_rl_repo", "/root/.axon_site/_ro/trn_rl_repo"):
    if os.path.isdir(_p) and _p not in sys.path:
        sys.path.append(_p)

import concourse.bass as bass
import concourse.mybir as mybir
import concourse.tile as tile
from concourse import bacc
from concourse.bass_utils import run_bass_kernel_spmd

# ---------------------------------------------------------------------------
# ACT table-set pinning: this kernel's only activations are Identity, Exp and
# Ln, all present in the single "natural_log_exp_and_others" set.  The stock
# per-function chooser maps Exp to "exp_and_others" and Ln to the natural-log
# set, which alternates ACT_TABLE_LOADs (~2.6us each) every attention chunk.
# Restrict the function->set map so every activation resolves to the one set
# and exactly one table load is emitted.
import concourse.hw_specs as _hw_specs

_orig_get_activation_tables = _hw_specs.get_activation_tables


def _pinned_activation_tables(module_arch):
    t = _orig_get_activation_tables(module_arch)
    pin = {
        mybir.ActivationFunctionType.Exp,
        mybir.ActivationFunctionType.Ln,
        mybir.ActivationFunctionType.Identity,
        mybir.ActivationFunctionType.Copy,
    }
    out = {}
    for name, fns in t.items():
        if name == "natural_log_exp_and_others":
            out[name] = set(fns)
        else:
            out[name] = set(fns) - pin
    return out


bacc.get_activation_tables = _pinned_activation_tables

B = 2
L = 2048
D_MODEL = 1024
NHEAD = 16
D_HEAD = 64
SCALE = 1.0 / float(np.sqrt(D_HEAD))
N_CORES = 8
TP = 4                      # tensor-parallel group size (heads split)
HEADS_PER_CORE = NHEAD // TP          # 4
D_LOCAL = HEADS_PER_CORE * D_HEAD     # 256
N_PAIRS = HEADS_PER_CORE // 2         # 2 head-pairs per core
KT = D_MODEL // 128                   # 8 contraction tiles for projections
LT = L // 128                         # 16 L tiles
NCH = L // 512                        # 4 Lq chunks of 512

F32 = mybir.dt.float32
# PE matmul dtype: float32r = single-pass fp32 (full speed, slightly
# reduced multiply precision); float32 = exact two-pass (4x slower).
_MM_DT_NAME = os.environ.get("KERNEL_MM_DT", "f32r")
MM_DT = {"f32": mybir.dt.float32, "f32r": mybir.dt.float32r}[_MM_DT_NAME]
MMD = MM_DT  # dtype of SBUF tiles consumed by the main matmuls
# attention operands (qT/kT/v_aug/exp-probs) in bf16: same PE cycles/row as
# fp32r but fast-weight-load halves LDWEIGHTS, and SBUF footprint halves
ATT_DT = {"bf16": mybir.dt.bfloat16, "f32r": mybir.dt.float32r,
          "f32": mybir.dt.float32}[os.environ.get("KERNEL_ATT_DT", "bf16")]
# projection inputs (x, Wq/Wk/Wv, bv) in bf16: halves the HBM load (which
# paces the kernel head) at ~2e-3 relative error
PROJ_DT = {"bf16": mybir.dt.bfloat16, "f32r": mybir.dt.float32r,
           "f32": mybir.dt.float32}[os.environ.get("KERNEL_PROJ_DT", "bf16")]

# v_aug stationary layout (one tile per (pair, Lk-tile), [128, 193]):
#   cols 0:64    = v(even head)          -> even window cols 0:128
#   col  64      = ones (even denom)
#   cols 65:129  = zeros
#   col  97      = ones (odd denom; odd window col index 32)
#   cols 129:193 = v(odd head)           -> odd window cols 65:193
VAUG_W = 193


def _to_proj(x):
    if PROJ_DT is mybir.dt.bfloat16:
        import ml_dtypes

        return np.ascontiguousarray(np.asarray(x, np.float32).astype(ml_dtypes.bfloat16))
    return _round_fp32r(x)


def _round_fp32r(x):
    """Round fp32 to the fp32r grid (11-bit mantissa; low 12 bits zero) so a
    plain same-dtype DMA feeds PE-exact fp32r values."""
    if MM_DT is not mybir.dt.float32r:
        return np.ascontiguousarray(x, dtype=np.float32)
    b = np.ascontiguousarray(x, dtype=np.float32).view(np.uint32).astype(np.uint64)
    r = ((b + 0x7FF + ((b >> 12) & 1)) >> 12 << 12) & 0xFFFFFFFF
    return r.astype(np.uint32).view(np.float32)


def build_kernel():
    nc = bacc.Bacc("TRN2", target_bir_lowering=False, debug=False)

    xT = nc.dram_tensor("xT", [D_MODEL, L], PROJ_DT, kind="ExternalInput").ap()
    wq = nc.dram_tensor("wq", [D_MODEL, D_LOCAL], PROJ_DT, kind="ExternalInput").ap()
    wk = nc.dram_tensor("wk", [D_MODEL, D_LOCAL], PROJ_DT, kind="ExternalInput").ap()
    wv = nc.dram_tensor("wv", [D_MODEL, D_LOCAL], PROJ_DT, kind="ExternalInput").ap()
    wo = nc.dram_tensor("wo", [D_LOCAL, D_MODEL], MMD, kind="ExternalInput").ap()
    bq = nc.dram_tensor("bq", [D_LOCAL], F32, kind="ExternalInput").ap()
    bk = nc.dram_tensor("bk", [D_LOCAL], F32, kind="ExternalInput").ap()
    bv = nc.dram_tensor("bv", [D_LOCAL], PROJ_DT, kind="ExternalInput").ap()
    bo = nc.dram_tensor("bo", [D_MODEL], MMD, kind="ExternalInput").ap()
    out_p = nc.dram_tensor("out_p", [L, D_MODEL], F32, kind="ExternalOutput").ap()

    Exp = mybir.ActivationFunctionType.Exp
    Ln = mybir.ActivationFunctionType.Ln

    with tile.TileContext(nc) as tc:
        with (
            tc.tile_pool(name="persist", bufs=1) as persist,
            tc.tile_pool(name="exp_pool", bufs=3) as exp_pool,
            tc.tile_pool(name="recip_pool", bufs=2) as recip_pool,
            tc.tile_pool(name="bcs_pool", bufs=2) as bcs_pool,
            tc.tile_pool(name="out_pool", bufs=4) as out_pool,
            tc.tile_pool(name="ps_st", bufs=2, space="PSUM") as ps_st,
            tc.tile_pool(name="ps_ctx", bufs=4, space="PSUM") as ps_ctx,
        ):
            # ---- constants / weights (sync queue: v-path + xT first; the
            # scalar queue carries the rest in parallel) ----
            ones_sb = persist.tile([128, 128], MMD)
            nc.vector.memset(ones_sb[:].bitcast(F32), 1.0)
            ones_pj = persist.tile([1, 128], PROJ_DT)
            nc.vector.memset(
                ones_pj[:].bitcast(F32) if PROJ_DT == mybir.dt.float32r
                else ones_pj[:],
                1.0,
            )
            bv_sb = persist.tile([1, D_LOCAL], PROJ_DT)
            nc.sync.dma_start(bv_sb[:], bv.rearrange("(o n) -> o n", o=1))
            wv_sb = persist.tile([128, KT, D_LOCAL], PROJ_DT)
            nc.sync.dma_start(wv_sb[:], wv.rearrange("(k p) n -> p k n", p=128))
            xT_sb = persist.tile([128, KT, L], PROJ_DT)
            for k in range(KT):
                nc.sync.dma_start(xT_sb[:, k, :], xT[k * 128 : (k + 1) * 128, :])
            bq_sb = persist.tile([128, D_LOCAL // 128], F32)
            nc.scalar.dma_start(bq_sb[:], bq.rearrange("(m p) -> p m", p=128))
            bk_sb = persist.tile([128, D_LOCAL // 128], F32)
            nc.scalar.dma_start(bk_sb[:], bk.rearrange("(m p) -> p m", p=128))
            bo_sb = persist.tile([1, D_MODEL], MMD)
            nc.scalar.dma_start(bo_sb[:], bo.rearrange("(o n) -> o n", o=1))
            wq_sb = persist.tile([128, KT, D_LOCAL], PROJ_DT)
            nc.scalar.dma_start(wq_sb[:], wq.rearrange("(k p) n -> p k n", p=128))
            wk_sb = persist.tile([128, KT, D_LOCAL], PROJ_DT)
            nc.scalar.dma_start(wk_sb[:], wk.rearrange("(k p) n -> p k n", p=128))
            wo_sb = persist.tile([128, N_PAIRS, D_MODEL], MMD)
            nc.scalar.dma_start(wo_sb[:], wo.rearrange("(k p) n -> p k n", p=128))

            qT_sb = persist.tile([128, N_PAIRS, L], ATT_DT)
            kT_sb = persist.tile([128, N_PAIRS, L], ATT_DT)
            vaug = persist.tile([128, N_PAIRS, LT, VAUG_W], ATT_DT)
            ctxT_sb = persist.tile([128, N_PAIRS, L], MMD)

            _vm = (lambda ap: ap.bitcast(F32)) if ATT_DT == mybir.dt.float32r else (
                lambda ap: ap
            )
            nc.vector.memset(_vm(vaug[:, :, :, 64:129]), 0.0)
            nc.vector.memset(_vm(vaug[:, :, :, 64:65]), 1.0)
            nc.vector.memset(_vm(vaug[:, :, :, 97:98]), 1.0)

            # ---- v projection (natural layout; bias via K=1 rank-1) ----
            def emit_v(lt):
                ps = ps_ctx.tile([128, D_LOCAL], F32, tag="ctx")
                for k in range(KT):
                    nc.tensor.matmul(
                        ps[:],
                        xT_sb[:, k, lt * 128 : (lt + 1) * 128],
                        wv_sb[:, k, :],
                        start=(k == 0),
                        stop=False,
                    )
                nc.tensor.matmul(
                    ps[:], ones_pj[0:1, 0:128], bv_sb[0:1, :], start=False, stop=True
                )
                for p in range(N_PAIRS):
                    nc.vector.tensor_copy(
                        vaug[:, p, lt, 0:64], ps[:, p * 128 : p * 128 + 64]
                    )
                    nc.vector.tensor_copy(
                        vaug[:, p, lt, 129:193],
                        ps[:, p * 128 + 64 : p * 128 + 128],
                    )

            # ---- one qT/kT projection group: tensor t (0=q, 1=k), pair m,
            # Lq chunk c ----
            def emit_qk(t, m, c):
                w_sb, b_sb, dst = ((wq_sb, bq_sb, qT_sb), (wk_sb, bk_sb, kT_sb))[t]
                ps = ps_ctx.tile([128, 512], F32, tag="ctx")
                for k in range(KT):
                    nc.tensor.matmul(
                        ps[:],
                        w_sb[:, k, m * 128 : (m + 1) * 128],
                        xT_sb[:, k, c * 512 : (c + 1) * 512],
                        start=(k == 0),
                        stop=(k == KT - 1),
                    )
                nc.vector.tensor_scalar_add(
                    dst[:, m, c * 512 : (c + 1) * 512], ps[:], b_sb[:, m : m + 1]
                )

            def emit_normalize(p, c, ctx_e, ctx_o):
                # softmax denominators sit at psum partition 64 (even head)
                # / 32 (odd head).  1/d = exp(-ln d) on the Scalar engine
                # (both functions in one pinned ACT table set), an
                # SBUF->SBUF DMA broadcasts each row across partitions, and
                # one DVE multiply per head fuses normalize with the
                # PSUM->SBUF evacuation into the packed ctxT tile.
                rt = recip_pool.tile([128, 512], F32, tag="rt")
                nc.vector.reciprocal(rt[64:65, :], ctx_e[64:65, :])
                nc.vector.reciprocal(rt[32:33, :], ctx_o[32:33, :])
                bcs = bcs_pool.tile([128, 512], F32, tag="bcs")
                nc.sync.dma_start(
                    bcs[0:64, :],
                    rt[64:65, :].unsqueeze(1).broadcast_to([1, 64, 512]),
                )
                nc.sync.dma_start(
                    bcs[64:128, :],
                    rt[32:33, :].unsqueeze(1).broadcast_to([1, 64, 512]),
                )
                sl = slice(c * 512, (c + 1) * 512)
                nc.vector.tensor_mul(
                    ctxT_sb[0:64, p, sl], ctx_e[0:64, :], bcs[0:64, :]
                )
                nc.vector.tensor_mul(
                    ctxT_sb[64:128, p, sl], ctx_o[64:128, :], bcs[64:128, :]
                )

            def emit_outproj_tile(c, idx):
                # one [128, 512] tile of out[L, D] for Lq chunk c; psum
                # recycled from the ctx pool.
                m = 4 * c + idx // 2
                n = idx % 2
                po = ps_ctx.tile([128, 512], F32, tag="ctx")
                for k in range(N_PAIRS):
                    nc.tensor.matmul(
                        po[:],
                        ctxT_sb[:, k, m * 128 : (m + 1) * 128],
                        wo_sb[:, k, n * 512 : (n + 1) * 512],
                        start=(k == 0),
                        stop=False,
                    )
                nc.tensor.matmul(
                    po[:],
                    ones_sb[0:1, 0:128],
                    bo_sb[0:1, n * 512 : (n + 1) * 512],
                    start=False,
                    stop=True,
                )
                ot = out_pool.tile([128, 512], F32, tag="ot")
                nc.vector.tensor_copy(ot[:], po[:])
                nc.sync.dma_start(
                    out_p[m * 128 : (m + 1) * 128, n * 512 : (n + 1) * 512], ot[:]
                )

            # ---- emission schedule ----
            # PE warm-up: dense dummy matmuls on resident data while the
            # input DMAs land, so the HAM clock-gate reaches 2.4 GHz before
            # the real projections start (~3.4us of sustained PE activity).
            warm = ps_st.tile([128, 1024], F32, tag="sT")
            for i in range(40):
                nc.tensor.matmul(
                    warm[:, 0:128],
                    ones_sb[0:1, 0:128],
                    ones_sb[0:1, 0:128],
                    start=(i == 0),
                    stop=(i == 39),
                )

            # head: v projection + pair-0 q/k projections
            for lt in range(LT):
                emit_v(lt)
            for t in range(2):
                for c in range(NCH):
                    emit_qk(t, 0, c)

            # attention, pair-outer; pair-1 projections are drip-fed into
            # pair-0's attention (PE slack under the ACT-paced exp), and the
            # fused out-projection drips into pair-1's attention.
            qk_todo = [(t, 1, c) for t in range(2) for c in range(NCH)]
            outproj_q = []
            pending = None
            for p in range(N_PAIRS):
                for c in range(NCH):
                    ctx_e = ps_ctx.tile([128, 512], F32, tag="ctx")
                    ctx_o = ps_ctx.tile([128, 512], F32, tag="ctx")
                    if pending is not None:
                        # normalize the previous chunk now: all of it runs
                        # on DVE/DMA, so it overlaps this chunk's matmuls
                        # without costing the PE or ACT pacers anything
                        prev_p, prev_c = pending[0], pending[1]
                        emit_normalize(*pending)
                        pending = None
                        if prev_p == N_PAIRS - 1:
                            outproj_q.extend((prev_c, idx) for idx in range(8))
                    for j in range(LT):
                        sT = ps_st.tile([128, 1024], F32, tag="sT")
                        nc.tensor.matmul(
                            sT[:, 0:512],
                            kT_sb[0:64, p, j * 128 : (j + 1) * 128],
                            qT_sb[0:64, p, c * 512 : (c + 1) * 512],
                            start=True,
                            stop=True,
                        )
                        nc.tensor.matmul(
                            sT[:, 512:1024],
                            kT_sb[64:128, p, j * 128 : (j + 1) * 128],
                            qT_sb[64:128, p, c * 512 : (c + 1) * 512],
                            start=True,
                            stop=True,
                        )
                        et = exp_pool.tile([128, 1024], ATT_DT, tag="et")
                        nc.scalar.activation(et[:], sT[:], Exp, scale=SCALE)
                        nc.tensor.matmul(
                            ctx_e[:],
                            vaug[:, p, j, 0:128],
                            et[:, 0:512],
                            start=(j == 0),
                            stop=(j == LT - 1),
                        )
                        nc.tensor.matmul(
                            ctx_o[:],
                            vaug[:, p, j, 65:193],
                            et[:, 512:1024],
                            start=(j == 0),
                            stop=(j == LT - 1),
                        )
                        if p == 0 and j in (6, 12) and qk_todo:
                            emit_qk(*qk_todo.pop(0))
                        if j % 2 == 0 and j >= 8 and outproj_q:
                            emit_outproj_tile(*outproj_q.pop(0))
                    pending = (p, c, ctx_e, ctx_o)
            emit_normalize(*pending)
            outproj_q.extend((NCH - 1, idx) for idx in range(8))
            for tile_ref in outproj_q:
                emit_outproj_tile(*tile_ref)

    nc.compile()
    return nc


_NC = None
LAST_RESULTS = None


def _get_nc():
    global _NC
    if _NC is None:
        _NC = build_kernel()
    return _NC


def kernel(x, Wq, bq, Wk, bk, Wv, bv, Wo, bo):
    global LAST_RESULTS
    x = np.asarray(x, dtype=np.float32)
    Wq = np.asarray(Wq, dtype=np.float32)
    Wk = np.asarray(Wk, dtype=np.float32)
    Wv = np.asarray(Wv, dtype=np.float32)
    Wo = np.asarray(Wo, dtype=np.float32)
    bq = np.asarray(bq, dtype=np.float32)
    bk = np.asarray(bk, dtype=np.float32)
    bv = np.asarray(bv, dtype=np.float32)
    bo = np.asarray(bo, dtype=np.float32)

    nc = _get_nc()

    xTb = [_to_proj(x[b].T) for b in range(B)]
    zeros_bo = np.zeros_like(bo)
    in_maps = []
    for c in range(N_CORES):
        b, tp = divmod(c, TP)
        sl = slice(tp * D_LOCAL, (tp + 1) * D_LOCAL)
        in_maps.append(
            {
                "xT": xTb[b],
                "wq": _to_proj(Wq[:, sl]),
                "wk": _to_proj(Wk[:, sl]),
                "wv": _to_proj(Wv[:, sl]),
                "wo": _round_fp32r(Wo[sl, :]),
                "bq": np.ascontiguousarray(bq[sl]),
                "bk": np.ascontiguousarray(bk[sl]),
                "bv": _to_proj(bv[sl]),
                "bo": _round_fp32r(bo) if tp == 0 else zeros_bo,
            }
        )

    res = run_bass_kernel_spmd(nc, in_maps, core_ids=list(range(N_CORES)))
    LAST_RESULTS = res

    out = np.empty((B, L, D_MODEL), dtype=np.float32)
    for b in range(B):
        acc = res.results[b * TP]["out_p"].astype(np.float32)
        for tp in range(1, TP):
            acc = acc + res.results[b * TP + tp]["out_p"]
        out[b] = acc
    return out
